# revision 7
# baseline (speedup 1.0000x reference)
"""Trainium2 Bass kernel for nn_Conv2d_layer_36584531427330.

Computes: conv_transpose2d(x, w, stride=2) -> depthwise 4x4 FIR ([1,3,3,1]/8
separable, gain 4) -> +bias -> leaky_relu(0.2) * sqrt(2).
  x: (32, 512, 32, 32) f32 -> out: (32, 256, 64, 64) f32

Strategy (data-parallel over batch, 4 images per core on 8 cores):
- The stride-2 transposed conv is decomposed into 4 output-parity phases
  (EE/EO/OE/OO) with 4/2/2/1 taps; each tap is a [K=128ic x M=128oc x N]
  matmul accumulated in PSUM (fp16 operands, fp32 accumulate).
- The separable FIR runs on the vector engine as fused scalar_tensor_tensor
  MACs over the phase grids: per dim, out = 1*A + 3*B + 3*C + 1*D with the
  global 1/16 (and the conv gain and sqrt2) folded into the PSUM-evict scale.
- PSUM->SBUF eviction (with scale, ->fp16) runs on the scalar engine, which
  also applies the final bias + leaky relu (Lrelu activation) to fp32.
Output layout interleaving (phase -> natural y/x order) happens in the final
activation's strided write; DMA transfers are fully contiguous.
"""

import numpy as np

import concourse.bass as bass
from concourse import bacc
import concourse.mybir as mybir
import concourse.tile as tile
from concourse.bass_utils import run_bass_kernel_spmd

N_CORES = 8
B, IC, OC, K = 32, 512, 256, 3
BPC = B // N_CORES          # images per core
ICC = IC // 128             # ic chunks
SQRT2 = 1.4142135623730951
PLANE = 34 * 34 + 34  # padded plane + overrun tail
GAIN = 1.0 / np.sqrt(IC * K * K)
S_EVICT = float(GAIN * SQRT2 / 16.0)

F16 = mybir.dt.float16
F32 = mybir.dt.float32

# phase: (name, nrows, ncols, taps[(dy,dx)], row_pad_off, col_pad_off)
PHASES = [
    ("EE", 33, 33, [(0, 0), (0, 2), (2, 0), (2, 2)], 0, 0),
    ("EO", 33, 32, [(0, 1), (2, 1)], 0, 1),
    ("OE", 32, 33, [(1, 0), (1, 2)], 1, 0),
    ("OO", 32, 32, [(1, 1)], 1, 1),
]


def _row_chunks(nrows):
    # contiguous row spans with rows*34 <= 512 (PSUM bank / moving-dim limit)
    out = []
    r = 0
    while r < nrows:
        n = min(11, nrows - r)
        out.append((r, n))
        r += n
    return out


def _w_off(icc, dy, dx, half):
    return ((((icc * 3 + dy) * 3 + dx) * 2) + half) * 128


def _build_nc():
    nc = bacc.Bacc(None, target_bir_lowering=False)
    # each image plane is zero-padded 34x34, flattened, plus a 34-elem zero
    # tail so contiguous tap windows of the bottom row-chunk stay in bounds
    xp = nc.dram_tensor("xp", [BPC, IC, PLANE], F16, kind="ExternalInput")
    wt = nc.dram_tensor("wt", [128, ICC * 3 * 3 * 2 * 128], F16, kind="ExternalInput")
    bias2 = nc.dram_tensor("bias2", [128, 2], F16, kind="ExternalInput")
    out = nc.dram_tensor("out", [BPC, OC, 64, 64], F32, kind="ExternalOutput")

    add, mult = mybir.AluOpType.add, mybir.AluOpType.mult

    with tile.TileContext(nc) as tc:
        with (
            tc.tile_pool(name="const", bufs=1) as cpool,
            tc.tile_pool(name="pers", bufs=1) as ppool,
            tc.tile_pool(name="xin", bufs=2) as xpool,
            tc.tile_pool(name="y1", bufs=2) as y1pool,
            tc.tile_pool(name="z", bufs=2) as zpool,
            tc.tile_pool(name="scratch", bufs=3) as spool,
            tc.tile_pool(name="outp", bufs=2) as opool,
            tc.tile_pool(name="psum", bufs=6, space="PSUM") as pspool,
        ):
            w_sb = cpool.tile([128, ICC * 3 * 3 * 2 * 128], F16, name="w_sb")
            bias_sb = cpool.tile([128, 2], F16, name="bias_sb")
            nc.sync.dma_start(w_sb[:], wt[:])
            nc.sync.dma_start(bias_sb[:], bias2[:])

            # persistent padded phase tiles (manual double buffer, borders
            # zeroed once and only interiors ever rewritten)
            pers = {}
            for nm, shp in (
                ("EO", [128, 33, 34]),
                ("OE", [128, 34, 33]),
                ("OO", [128, 34, 34]),
                ("zO", [128, 34, 2, 32]),
            ):
                pers[nm] = [
                    ppool.tile(shp, F16, name=f"{nm}{i}") for i in range(2)
                ]
            for i in range(2):
                eo, oe, oo, zo = pers["EO"][i], pers["OE"][i], pers["OO"][i], pers["zO"][i]
                nc.gpsimd.memset(eo[:, :, 0:1], 0.0)
                nc.gpsimd.memset(eo[:, :, 33:34], 0.0)
                nc.gpsimd.memset(oe[:, 0:1, :], 0.0)
                nc.gpsimd.memset(oe[:, 33:34, :], 0.0)
                nc.gpsimd.memset(oo[:, 0:1, :], 0.0)
                nc.gpsimd.memset(oo[:, 33:34, :], 0.0)
                nc.gpsimd.memset(oo[:, 1:33, 0:1], 0.0)
                nc.gpsimd.memset(oo[:, 1:33, 33:34], 0.0)
                nc.gpsimd.memset(zo[:, 0:1], 0.0)
                nc.gpsimd.memset(zo[:, 33:34], 0.0)

            for img in range(BPC):
                x_sb = xpool.tile([128, ICC, PLANE], F16, name="x_sb", tag="x_sb")
                nc.sync.dma_start(
                    x_sb[:],
                    xp[img].rearrange("(c p) f -> p c f", p=128),
                )
                for half in range(2):
                    slab = img * 2 + half
                    buf = slab % 2
                    EO_t, OE_t, OO_t = (
                        pers["EO"][buf], pers["OE"][buf], pers["OO"][buf],
                    )
                    zO = pers["zO"][buf]
                    EE_t = y1pool.tile([128, 33, 33], F16, name="EE_t", tag="EE")
                    ph_tiles = {"EE": EE_t, "EO": EO_t, "OE": OE_t, "OO": OO_t}

                    # ---- stage 1: phase matmuls + scaled eviction ----
                    for nm, nrows, ncols, taps, ro, co in PHASES:
                        dstt = ph_tiles[nm]
                        for r0, rn in _row_chunks(nrows):
                            N = rn * 34
                            ps = pspool.tile([128, 11 * 34], F32, name="ps", tag="ps")
                            nmm = len(taps) * ICC
                            kk = 0
                            for dy, dx in taps:
                                ey = -1 if dy == 2 else 0
                                ex = -1 if dx == 2 else 0
                                for icc in range(ICC):
                                    st = (r0 + ey + 1) * 34 + (ex + 1)
                                    nc.tensor.matmul(
                                        ps[:, :N],
                                        lhsT=w_sb[:, _w_off(icc, dy, dx, half):
                                                  _w_off(icc, dy, dx, half) + 128],
                                        rhs=x_sb[:, icc, st:st + N],
                                        start=(kk == 0),
                                        stop=(kk == nmm - 1),
                                    )
                                    kk += 1
                            psv = ps[:, :N].rearrange("p (r c) -> p r c", c=34)
                            nc.scalar.mul(
                                dstt[:, ro + r0:ro + r0 + rn, co:co + ncols],
                                psv[:, :, 0:ncols],
                                S_EVICT,
                            )

                    # ---- FIR-H (vector engine) ----
                    zE = zpool.tile([128, 33, 2, 32], F16, name="zE", tag="zE")
                    # y-even rows: E-x = EE (33 rows), O-x = EO (padded cols)
                    # y-odd rows:  E-x = OE rows 1:33, O-x = OO rows 1:33
                    for (Ex, Ox, rs, zt, zr) in (
                        (EE_t[:, :, :], EO_t[:, :, :], 33, zE, slice(0, 33)),
                        (OE_t[:, 1:33, :], OO_t[:, 1:33, :], 32, zO, slice(1, 33)),
                    ):
                        s1 = spool.tile([128, 33, 32], F16, name="s1", tag="hs1")
                        s2 = spool.tile([128, 33, 32], F16, name="s2", tag="hs2")
                        s1v, s2v = s1[:, :rs, :], s2[:, :rs, :]
                        # even x: z = 3*E[r] + Op[r]  +  3*Op[r+1] + E[r+1]
                        nc.vector.scalar_tensor_tensor(
                            s1v, Ex[:, :, 0:32], 3.0, Ox[:, :, 0:32], mult, add)
                        nc.vector.scalar_tensor_tensor(
                            s2v, Ox[:, :, 1:33], 3.0, Ex[:, :, 1:33], mult, add)
                        nc.vector.tensor_tensor(zt[:, zr, 0, :], s1v, s2v, add)
                        # odd x: z = 3*Op[r+1] + E[r]  +  3*E[r+1] + Op[r+2]
                        s3 = spool.tile([128, 33, 32], F16, name="s3", tag="hs1")
                        s4 = spool.tile([128, 33, 32], F16, name="s4", tag="hs2")
                        s3v, s4v = s3[:, :rs, :], s4[:, :rs, :]
                        nc.vector.scalar_tensor_tensor(
                            s3v, Ox[:, :, 1:33], 3.0, Ex[:, :, 0:32], mult, add)
                        nc.vector.scalar_tensor_tensor(
                            s4v, Ex[:, :, 1:33], 3.0, Ox[:, :, 2:34], mult, add)
                        nc.vector.tensor_tensor(zt[:, zr, 1, :], s3v, s4v, add)

                    # ---- FIR-V (vector engine) ----
                    zEf = zE.rearrange("p r t c -> p r (t c)")
                    zOf = zO.rearrange("p r t c -> p r (t c)")
                    out_pre = opool.tile([128, 64, 64], F16, name="out_pre",
                                         tag="out_pre")
                    opr = out_pre.rearrange("p (q t) c -> p q t c", t=2)
                    v1 = spool.tile([128, 32, 64], F16, name="v1", tag="vs1")
                    v2 = spool.tile([128, 32, 64], F16, name="v2", tag="vs2")
                    nc.vector.scalar_tensor_tensor(
                        v1[:], zEf[:, 0:32], 3.0, zOf[:, 0:32], mult, add)
                    nc.vector.scalar_tensor_tensor(
                        v2[:], zOf[:, 1:33], 3.0, zEf[:, 1:33], mult, add)
                    nc.vector.scalar_tensor_tensor(opr[:, :, 0, :], v1[:], bias_sb[:, half:half + 1], v2[:], add, add)
                    v3 = spool.tile([128, 32, 64], F16, name="v3", tag="vs1")
                    v4 = spool.tile([128, 32, 64], F16, name="v4", tag="vs2")
                    nc.vector.scalar_tensor_tensor(
                        v3[:], zOf[:, 1:33], 3.0, zEf[:, 0:32], mult, add)
                    nc.vector.scalar_tensor_tensor(
                        v4[:], zEf[:, 1:33], 3.0, zOf[:, 2:34], mult, add)
                    nc.vector.scalar_tensor_tensor(opr[:, :, 1, :], v3[:], bias_sb[:, half:half + 1], v4[:], add, add)

                    # ---- leaky relu: max(0.2u, u) on DVE (exact) ----
                    lk = opool.tile([128, 64, 64], F16, name="lk", tag="lk")
                    nc.vector.scalar_tensor_tensor(
                        lk[:], out_pre[:], 0.2, out_pre[:], mult,
                        mybir.AluOpType.max)
                    # ---- convert to fp32 + x-deinterleave (scalar engine) ----
                    out_f32 = opool.tile([128, 64 * 64], F32, name="out_f32",
                                         tag="out_f32")
                    ofv = out_f32.rearrange("p (y r t) -> p y t r", y=64, r=32)
                    lkv = lk.rearrange("p y (t r) -> p y t r", t=2)
                    nc.scalar.copy(ofv, lkv)
                    nc.sync.dma_start(
                        out[img, half * 128:(half + 1) * 128]
                        .rearrange("o h w -> o (h w)"),
                        out_f32[:],
                    )
    nc.finalize()
    return nc


_NC_CACHE = None


def _get_nc():
    global _NC_CACHE
    if _NC_CACHE is None:
        _NC_CACHE = _build_nc()
    return _NC_CACHE


def _prep_inputs(x, weight, bias):
    x = np.asarray(x, dtype=np.float32)
    weight = np.asarray(weight, dtype=np.float32)
    bias = np.asarray(bias, dtype=np.float32)

    t = weight.reshape(2, 128, ICC, 128, 3, 3)       # (half, ocl, icc, icp, dy, dx)
    t = np.transpose(t, (3, 2, 4, 5, 0, 1))          # (icp, icc, dy, dx, half, ocl)
    wt_host = np.ascontiguousarray(t.reshape(128, -1)).astype(np.float16)

    bias2_host = np.ascontiguousarray(
        (bias * np.float32(SQRT2)).reshape(2, 128).T
    ).astype(np.float16)

    x16 = x.astype(np.float16)
    in_maps = []
    for c in range(N_CORES):
        xp_host = np.zeros((BPC, IC, PLANE), np.float16)
        pl = np.zeros((BPC, IC, 34, 34), np.float16)
        pl[:, :, 1:33, 1:33] = x16[c * BPC:(c + 1) * BPC]
        xp_host[:, :, :34 * 34] = pl.reshape(BPC, IC, -1)
        in_maps.append({"xp": xp_host, "wt": wt_host, "bias2": bias2_host})
    return in_maps


def _execute(x, weight, bias, trace=False):
    nc = _get_nc()
    in_maps = _prep_inputs(x, weight, bias)
    res = run_bass_kernel_spmd(nc, in_maps, core_ids=list(range(N_CORES)),
                               trace=trace)
    out = np.concatenate([r["out"] for r in res.results], axis=0)
    return out, res


def kernel(x, weight, bias):
    out, _ = _execute(x, weight, bias, trace=False)
    return out
